# revision 2
# baseline (speedup 1.0000x reference)
"""Trainium2 Bass kernel for nn_ConsistencyLoss — v3.

DFT factorization (per core: one batch row x 256 of 512 frequency bins):
  A) H[t, n] = frames @ (W * DFT_512)             fp8 DoubleRow (j-pairs)
  B) hhat[f, n] = sum_t H[t, n] e^{-2pi i f t/L}  L = 1536, fp8 DoubleRow
  C) ghat = Khat * hhat                            elementwise (Act/DVE/Pool)
  D) C[t', n] = sum_f ghat e^{+2pi i f (t'+511)/L} fp8 DoubleRow
  E) loss += |C|^2 via DVE bn_stats (host reconstructs sum of squares)

DoubleRow packs (cos, sin) twiddles as the two contraction halves of one fp8
matmul: out[M,N] = cos.T @ h + sin.T @ hp at 0.5 cycles/output-column.
All constant matrices are host-pre-rearranged so every DMA lands contiguous
(2-3KB per partition) and all loads are issued up front.
"""
import numpy as np
import ml_dtypes

N = 512
R = 128
Q = 4
T = 1025
TP = 1152            # frames padded to 9*128
LDFT = 1536          # 12*128
NB = 256             # bins per core
FCH = 12             # f chunks of 128
TCH = 9              # t chunks of 128
B = 4

F8 = ml_dtypes.float8_e4m3
BF16 = ml_dtypes.bfloat16


# ---------------------------------------------------------------- host prep
def _pairify(mat, ncols):
    """[512, ncols] -> [2, 128, 2, ncols] with j = pair*256 + slot*128 + p."""
    return mat.reshape(2, 2, 128, ncols).transpose(0, 2, 1, 3)


def _build_host_constants(window, alpha_real, alpha_imag):
    alpha = alpha_real.astype(np.complex128) + 1j * alpha_imag.astype(np.complex128)
    n_idx = np.arange(N)
    q_idx = np.arange(-(Q - 1), Q)
    phase = np.exp(1j * (2 * np.pi / N) * np.outer(n_idx, q_idx))
    K = phase @ alpha                                 # (512, 1023)
    Khat = np.fft.fft(K, LDFT, axis=1) / LDFT         # (512, 1536)

    # scale khat so ghat stays clear of fp8 subnormals; host divides by sc^2
    rms = np.sqrt(np.mean(np.abs(Khat) ** 2))
    sc = float(2.0 ** np.round(np.log2(0.25 / rms)))
    Khat = Khat * sc

    W = window.astype(np.float64)
    j = np.arange(N)
    wdfts, khats = [], []
    for half in range(2):
        ns = np.arange(half * NB, half * NB + NB)
        ang = 2 * np.pi * np.outer(j, ns) / N
        wd = np.concatenate([
            W[:, None] * np.cos(ang),
            -W[:, None] * np.sin(ang),
        ], axis=1)                                    # (512, 512)
        wdfts.append(_pairify(wd, 512).astype(F8))    # (2, 128, 2, 512)
        ks = Khat[half * NB: half * NB + NB]          # (256, 1536)
        kh = np.concatenate([ks.real.T, ks.imag.T, -ks.imag.T], axis=1)
        khats.append(kh.astype(BF16))                 # (1536, 768)

    t_idx = np.arange(TP)
    f_idx = np.arange(LDFT)
    angl = 2 * np.pi * np.outer(t_idx, f_idx) / LDFT
    C1, S1 = np.cos(angl), np.sin(angl)
    # e1[fi, p, kt, s, c]: s0 = cos, s1 = sin of 2pi*t*f/L, t = kt*128+p,
    # f = fi*128 + c  (partition-contiguous: 2304 B per partition).
    e1 = np.empty((FCH, 128, TCH, 2, 128), dtype=F8)
    for fi in range(FCH):
        cb = C1[:, fi * 128:(fi + 1) * 128].reshape(TCH, 128, 128)
        sb = S1[:, fi * 128:(fi + 1) * 128].reshape(TCH, 128, 128)
        e1[fi, :, :, 0, :] = cb.transpose(1, 0, 2)
        e1[fi, :, :, 1, :] = sb.transpose(1, 0, 2)

    m_idx = 511 + np.arange(TP)
    ang2 = 2 * np.pi * np.outer(f_idx, m_idx) / LDFT
    C2, S2 = np.cos(ang2), np.sin(ang2)
    C2[:, T:] = 0.0
    S2[:, T:] = 0.0
    # e2[tc, p, fk, s, c]: trig of 2pi*f*(tc*128+c+511)/L, f = fk*128+p
    e2 = np.empty((TCH, 128, FCH, 2, 128), dtype=F8)
    for tc in range(TCH):
        cb = C2[:, tc * 128:(tc + 1) * 128].reshape(FCH, 128, 128)
        sb = S2[:, tc * 128:(tc + 1) * 128].reshape(FCH, 128, 128)
        e2[tc, :, :, 0, :] = cb.transpose(1, 0, 2)
        e2[tc, :, :, 1, :] = sb.transpose(1, 0, 2)
    return wdfts, e1, e2, khats, sc


def _build_frames(waveform):
    pad = np.pad(waveform.astype(np.float32), ((0, 0), (N // 2, N // 2)),
                 mode="reflect")
    Bn = waveform.shape[0]
    sb, se = pad.strides
    view = np.lib.stride_tricks.as_strided(
        pad, shape=(Bn, N, T), strides=(sb, se, R * se), writeable=False)
    out = np.zeros((Bn, N, TP), dtype=np.float32)
    out[:, :, :T] = view
    # [Bn, 2, 128, 2, TP] fp8, j-pair layout for DoubleRow stage A
    return np.stack([_pairify(out[b], TP) for b in range(Bn)]).astype(F8)


# ---------------------------------------------------------------- bass kernel
_CACHE = {}


def _build_nc():
    import concourse.bass as bass
    import concourse.mybir as mybir
    import concourse.tile as tile
    from concourse import bacc
    from concourse.bass import ts
    from contextlib import ExitStack

    f32 = mybir.dt.float32
    bf16 = mybir.dt.bfloat16
    f8 = mybir.dt.float8e4
    DR = mybir.MatmulPerfMode.DoubleRow

    nc = bacc.Bacc("TRN2", target_bir_lowering=False, debug=False)

    framesT = nc.dram_tensor("framesT", [2, 128, 2, TP], f8, kind="ExternalInput")
    wdft = nc.dram_tensor("wdft", [2, 128, 2, 512], f8, kind="ExternalInput")
    e1 = nc.dram_tensor("e1", [FCH, 128, TCH, 2, 128], f8, kind="ExternalInput")
    e2 = nc.dram_tensor("e2", [TCH, 128, FCH, 2, 128], f8, kind="ExternalInput")
    khat = nc.dram_tensor("khat", [LDFT, 768], bf16, kind="ExternalInput")
    accs_d = nc.dram_tensor("accs", [128, TCH * 6], f32, kind="ExternalOutput")

    with tile.TileContext(nc) as tc, ExitStack() as ctx:
        const = ctx.enter_context(tc.tile_pool(name="const", bufs=1))
        hpool = ctx.enter_context(tc.tile_pool(name="h", bufs=1))
        gpool = ctx.enter_context(tc.tile_pool(name="g", bufs=1))
        tmps = ctx.enter_context(tc.tile_pool(name="tmps", bufs=3))
        # 8 PSUM banks: stage B pipelines 5 deep; stages A and D share 3
        psB = ctx.enter_context(tc.tile_pool(name="psB", bufs=5, space="PSUM"))
        psD = ctx.enter_context(tc.tile_pool(name="psD", bufs=3, space="PSUM"))

        # ---- all constant loads issued up front, in consumption order
        fr_t, wd_t = [], []
        for pair in range(2):
            t1 = const.tile([128, 2, TP], f8, tag=f"fr{pair}")
            nc.sync.dma_start(t1[:], framesT[pair])
            fr_t.append(t1)
            t2 = const.tile([128, 2, 512], f8, tag=f"wd{pair}")
            nc.sync.dma_start(t2[:], wdft[pair])
            wd_t.append(t2)
        e1_t = [None] * FCH
        kh_t = [None] * FCH
        e2_t = [None] * TCH

        def _load_e1(fi):
            t3 = const.tile([128, TCH, 2, 128], f8, tag=f"e1_{fi}")
            nc.sync.dma_start(t3[:], e1[fi])
            e1_t[fi] = t3
            t4 = const.tile([128, 768], bf16, tag=f"kh{fi}")
            nc.sync.dma_start(t4[:], khat[ts(fi, 128), :])
            kh_t[fi] = t4

        def _load_e2(tc_i):
            t5 = const.tile([128, FCH, 2, 128], f8, tag=f"e2_{tc_i}")
            nc.sync.dma_start(t5[:], e2[tc_i])
            e2_t[tc_i] = t5

        # interleave by first-use time: e1/kh pace stage B; e2[0..2] arrive
        # mid-stream for the early D-partials; the rest trail just-in-time
        # for stage D so they never delay e1
        for fi in range(6):
            _load_e1(fi)
        for tc_i in range(3):
            _load_e2(tc_i)
        for fi in range(6, FCH):
            _load_e1(fi)
        for tc_i in range(3, TCH):
            _load_e2(tc_i)

        accs = const.tile([128, TCH, 6], f32, tag="accs")
        nc.vector.memset(accs[:], 0.0)

        # ---- stage A: H[t, n] -> h tiles [128, 2, 512] fp8
        #      slot0 = [Hre | Him], slot1 = [Him | -Hre]
        h_t = []
        for it in range(TCH):
            pA = psD.tile([128, 512], f32, tag="pD")
            for pair in range(2):
                nc.tensor.matmul(pA[:], fr_t[pair][:, :, ts(it, 128)],
                                 wd_t[pair][:], start=(pair == 0),
                                 stop=(pair == 1), perf_mode=DR)
            ht = hpool.tile([128, 2, 512], f8, tag=f"h{it}")
            nc.scalar.copy(ht[:, 0, :], pA[:])
            nc.vector.tensor_copy(ht[:, 1, 0:256], pA[:, 256:512])
            nc.vector.tensor_scalar_mul(ht[:, 1, 256:512], pA[:, 0:256], -1.0)
            h_t.append(ht)

        # ---- stage B + C (ghat = Khat * t-DFT of H), interleaved with the
        #      first 3 t'-chunks of stage D split into two half-contractions
        NSPLIT = 3
        g_t = [None] * FCH
        pD_part = [None] * NSPLIT

        def _stage_bc(fi):
            pB = psB.tile([128, 512], f32, tag="pB")
            for kt in range(TCH):
                nc.tensor.matmul(pB[:], e1_t[fi][:, kt], h_t[kt][:],
                                 start=(kt == 0), stop=(kt == TCH - 1),
                                 perf_mode=DR)
            kh = kh_t[fi]
            hb = tmps.tile([128, 512], bf16, tag="hb")
            nc.scalar.copy(hb[:], pB[:])
            gt = gpool.tile([128, 2, 512], f8, tag=f"g{fi}")
            t1 = tmps.tile([128, 256], bf16, tag="c1")
            t2 = tmps.tile([128, 256], bf16, tag="c2")
            t3 = tmps.tile([128, 256], bf16, tag="c3")
            t4 = tmps.tile([128, 256], bf16, tag="c4")
            # gre = hre*kre + him*(-kim);  gim = him*kre + hre*kim
            nc.vector.tensor_mul(t1[:], hb[:, 0:256], kh[:, 0:256])
            nc.vector.tensor_mul(t2[:], hb[:, 256:512], kh[:, 512:768])
            nc.vector.tensor_mul(t3[:], hb[:, 256:512], kh[:, 0:256])
            nc.vector.tensor_mul(t4[:], hb[:, 0:256], kh[:, 256:512])
            nc.vector.tensor_add(gt[:, 0, 0:256], t1[:], t2[:])
            nc.gpsimd.tensor_add(gt[:, 0, 256:512], t3[:], t4[:])
            # gb = [-gim | gre]
            nc.gpsimd.tensor_scalar_mul(gt[:, 1, 0:256], gt[:, 0, 256:512], -1.0)
            nc.scalar.copy(gt[:, 1, 256:512], gt[:, 0, 0:256])
            g_t[fi] = gt

        for fi in range(6):
            _stage_bc(fi)
        # first halves of D for tc 0..2 run on PE while stage C drains
        for tc_i in range(NSPLIT):
            pD = psD.tile([128, 512], f32, tag="pD")
            for fk in range(6):
                nc.tensor.matmul(pD[:], e2_t[tc_i][:, fk], g_t[fk][:],
                                 start=(fk == 0), stop=False, perf_mode=DR)
            pD_part[tc_i] = pD
        for fi in range(6, FCH):
            _stage_bc(fi)
        # stage B done with pB banks: first halves of D for tc 3..7 reuse them
        pB_part = [None] * 5
        for tc_i in range(3, 8):
            pD = psB.tile([128, 512], f32, tag="pB")
            for fk in range(6):
                nc.tensor.matmul(pD[:], e2_t[tc_i][:, fk], g_t[fk][:],
                                 start=(fk == 0), stop=False, perf_mode=DR)
            pB_part[tc_i - 3] = pD

        # ---- stage D + E: C[t', n]; |C|^2 via bn_stats (host reconstructs)
        for tc_i in range(8):
            pD = pD_part[tc_i] if tc_i < NSPLIT else pB_part[tc_i - 3]
            for fk in range(6, FCH):
                nc.tensor.matmul(pD[:], e2_t[tc_i][:, fk], g_t[fk][:],
                                 start=False, stop=(fk == FCH - 1),
                                 perf_mode=DR)
            nc.vector.bn_stats(accs[:, tc_i], pD[:])
        pD8 = psD.tile([128, 512], f32, tag="pD")
        for fk in range(FCH):
            nc.tensor.matmul(pD8[:], e2_t[8][:, fk], g_t[fk][:],
                             start=(fk == 0), stop=(fk == FCH - 1),
                             perf_mode=DR)
        nc.vector.bn_stats(accs[:, 8], pD8[:])

        nc.sync.dma_start(accs_d[:], accs[:].rearrange("p a b -> p (a b)"))

    nc.compile()
    return nc


def _make_runner(nc):
    """Cached shard-map runner: jit once, constants device-resident."""
    import jax
    from jax.experimental.shard_map import shard_map
    from jax.sharding import Mesh, NamedSharding, PartitionSpec
    from concourse import bass2jax
    import concourse.mybir as mybir

    bass2jax.install_neuronx_cc_hook()
    partition_name = nc.partition_id_tensor.name if nc.partition_id_tensor else None
    in_names, out_names, out_avals, zero_outs = [], [], [], []
    for alloc in nc.m.functions[0].allocations:
        if not isinstance(alloc, mybir.MemoryLocationSet):
            continue
        name = alloc.memorylocations[0].name
        if alloc.kind == "ExternalInput":
            if name != partition_name:
                in_names.append(name)
        elif alloc.kind == "ExternalOutput":
            shape = tuple(alloc.tensor_shape)
            dtype = mybir.dt.np(alloc.dtype)
            out_avals.append(jax.core.ShapedArray(shape, dtype))
            out_names.append(name)
            zero_outs.append(np.zeros(shape, dtype))
    n_params = len(in_names)
    n_outs = len(out_avals)
    all_names = list(in_names) + list(out_names)
    if partition_name is not None:
        all_names.append(partition_name)
    all_names = tuple(all_names)
    donate = tuple(range(n_params, n_params + n_outs))

    def _body(*args):
        operands = list(args)
        if partition_name is not None:
            operands.append(bass2jax.partition_id_tensor())
        outs = bass2jax._bass_exec_p.bind(
            *operands, out_avals=tuple(out_avals), in_names=all_names,
            out_names=tuple(out_names), lowering_input_output_aliases=(),
            sim_require_finite=True, sim_require_nnan=True, nc=nc)
        return tuple(outs)

    devices = jax.devices()[:8]
    mesh = Mesh(np.asarray(devices), ("core",))
    in_specs = (PartitionSpec("core"),) * (n_params + n_outs)
    out_specs = (PartitionSpec("core"),) * n_outs
    sharded = jax.jit(
        shard_map(_body, mesh=mesh, in_specs=in_specs,
                  out_specs=out_specs, check_rep=False),
        donate_argnums=donate, keep_unused=True)
    sharding = NamedSharding(mesh, PartitionSpec("core"))
    dev_cache = {}

    def run(in_maps, resident_names=()):
        import jax as _jax
        args = []
        for nm in in_names:
            if nm in dev_cache:
                args.append(dev_cache[nm])
                continue
            arr = np.concatenate([np.asarray(m[nm]) for m in in_maps], axis=0)
            if nm in resident_names:
                dev_cache[nm] = _jax.device_put(arr, sharding)
                args.append(dev_cache[nm])
            else:
                args.append(arr)
        for z in zero_outs:
            args.append(np.zeros((8 * z.shape[0], *z.shape[1:]), z.dtype))
        out_arrs = sharded(*args)
        return [{nm: np.asarray(out_arrs[i]).reshape(8, *out_avals[i].shape)[c]
                 for i, nm in enumerate(out_names)} for c in range(8)]

    return run


def kernel(waveform, window, alpha_real, alpha_imag):
    waveform = np.asarray(waveform)
    window = np.asarray(window)
    alpha_real = np.asarray(alpha_real)
    alpha_imag = np.asarray(alpha_imag)

    if "nc" not in _CACHE:
        _CACHE["nc"] = _build_nc()
    nc = _CACHE["nc"]

    ckey = (window.tobytes(), alpha_real.tobytes(), alpha_imag.tobytes())
    if _CACHE.get("ckey") != ckey:
        _CACHE["consts"] = _build_host_constants(window, alpha_real, alpha_imag)
        _CACHE["ckey"] = ckey
        _CACHE.pop("runner", None)   # drop device-resident stale constants
    wdfts, e1, e2, khats, sc = _CACHE["consts"]
    framesT = _build_frames(waveform)

    in_maps = []
    for core in range(8):
        b, half = core // 2, core % 2
        in_maps.append({
            "framesT": framesT[b],
            "wdft": wdfts[half],
            "e1": e1,
            "e2": e2,
            "khat": khats[half],
        })

    if "runner" not in _CACHE:
        _CACHE["runner"] = _make_runner(nc)
    results = _CACHE["runner"](
        in_maps, resident_names=("wdft", "e1", "e2", "khat"))
    total = 0.0
    for core in range(8):
        st = results[core]["accs"].astype(np.float64).reshape(128, TCH, 6)
        ne, me, ve = st[..., 0], st[..., 1], st[..., 2]
        no, mo, vo = st[..., 3], st[..., 4], st[..., 5]
        total += (ne * me * me + ve + no * mo * mo + vo).sum()
    return np.float32(total / (B * T) / (sc * sc))
